# revision 19
# baseline (speedup 1.0000x reference)
"""BoundaryTransformerLayer on 8 Trainium2 NeuronCores — full on-device pipeline.

Strategy (data-parallel over points):
- Host computes the cheap projections x_q/x_k/x_v and the 3-channel position
  MLP front (t1n = relu(bn_p(g_p @ Wp1.T + bp1))) — tiny tensors, exact fp32.
- Each core uploads only its shard: kv rows (2 MB), x_qT (1 MB), t1nT (1 MB),
  packed gather indices (0.3 MB) and the small weights.  Total ~4.4 MB/core
  up, 2 MB/core down — the axon tunnel transfer is the wall-clock bottleneck.
- On device: AllGather assembles the full [k|v] gather table (shard-rotated so
  int16 sign-extended indices address all 65536 rows from a mid-table base),
  dma_gather pulls 16 neighbors/point in 640-row chunks, and the whole tail
  (position-MLP tail matmul, two global BatchNorms via tiny AllReduces,
  softmax over neighbors, weighted aggregation) runs on-core, streaming
  intermediates through DRAM spills.
"""
import sys

sys.path.insert(0, "/opt/trn_rl_repo")

import numpy as np
import ml_dtypes

import concourse.bass as bass
import concourse.mybir as mybir
import concourse.tile as tile
from concourse import bacc
from concourse.bass_utils import run_bass_kernel_spmd
from concourse import bass2jax as _b2j

# Content-addressed NEFF disk cache: neuronxcc output is a pure function of
# the BIR, so memoize it across processes (saves ~0.5s on warm reruns; a
# cache miss just falls through to the normal compile).
_orig_compile_bir = _b2j.compile_bir_kernel


def _cached_compile_bir(bir_json, tmpdir, neff_name="file.neff"):
    import hashlib, os
    try:
        key = hashlib.sha256(bir_json).hexdigest()
        cdir = "/tmp/bass_neff_cache"
        os.makedirs(cdir, exist_ok=True)
        cpath = os.path.join(cdir, key + ".neff")
        if os.path.exists(cpath):
            out = os.path.join(tmpdir, neff_name)
            with open(cpath, "rb") as fsrc, open(out, "wb") as fdst:
                fdst.write(fsrc.read())
            return out
        neff_path = _orig_compile_bir(bir_json, tmpdir, neff_name)
        tmp = cpath + f".tmp{os.getpid()}"
        with open(neff_path, "rb") as fsrc, open(tmp, "wb") as fdst:
            fdst.write(fsrc.read())
        os.replace(tmp, cpath)
        return neff_path
    except Exception:
        return _orig_compile_bir(bir_json, tmpdir, neff_name)


_b2j.compile_bir_kernel = _cached_compile_bir

bf16 = mybir.dt.bfloat16
f32 = mybir.dt.float32
i16 = mybir.dt.int16
AX = mybir.AxisListType.X
OP = mybir.AluOpType
AF = mybir.ActivationFunctionType

EPS = 1e-5

FULL = dict(N=65536, NS=16, NC=8, CHUNK=8192)
MINI = dict(N=2048, NS=16, NC=8, CHUNK=2048)

_cache = {}


def _build(cfg_key):
    if cfg_key in _cache:
        return _cache[cfg_key]
    cfg = FULL if cfg_key == "full" else MINI
    N, NS, NC, CHUNK = cfg["N"], cfg["NS"], cfg["NC"], cfg["CHUNK"]
    NPTS = N // NC                 # points per core
    T = NPTS * NS                  # pairs per core
    NCHUNK = T // CHUNK            # pair-chunks per core
    G = CHUNK // 512               # gathers per chunk
    GN = 640                       # idxs per gather (512 real + 128 pad)
    ICOL = GN // 16                # idx cols per gather
    NT = N * NS                    # global BN count
    PPC = CHUNK // NS              # points per chunk
    SUB = CHUNK // 512             # 512-wide matmul subtiles per chunk

    nc = bacc.Bacc(None, target_bir_lowering=False, debug=False, num_devices=NC)

    xT = nc.dram_tensor("xT", [64, NPTS], bf16, kind="ExternalInput")
    wkvb = nc.dram_tensor("wkvb", [65, 128], bf16, kind="ExternalInput")  # [Wk.T|Wv.T ; bk|bv]
    wqb = nc.dram_tensor("wqb", [65, 64], bf16, kind="ExternalInput")     # [Wq.T ; bq]
    t1nT = nc.dram_tensor("t1nT", [3, T], bf16, kind="ExternalInput")
    idx16 = nc.dram_tensor("idx16", [16, NCHUNK * G * ICOL], i16, kind="ExternalInput")
    w3 = nc.dram_tensor("w3", [4, 128], bf16, kind="ExternalInput")    # Wp2T x2 (row3=0)
    w64 = nc.dram_tensor("w64", [64, 8], bf16, kind="ExternalInput")   # Ww1T
    w8 = nc.dram_tensor("w8", [8, 64], bf16, kind="ExternalInput")     # Ww2T tiled x8
    b64 = nc.dram_tensor("b64", [64, 5], f32, kind="ExternalInput")    # bp2|bn0g|bn0b|bw2r|eps
    b8 = nc.dram_tensor("b8", [8, 4], f32, kind="ExternalInput")       # bw1|bn1g|bn1b|eps
    b128 = nc.dram_tensor("b128", [128, 1], f32, kind="ExternalInput")  # bp2 x2
    outT = nc.dram_tensor("outT", [64, NPTS], bf16, kind="ExternalOutput")

    with tile.TileContext(nc) as tc:
        with (
            tc.tile_pool(name="dram", bufs=1, space="DRAM") as dram,
            tc.tile_pool(name="prm", bufs=1) as prm,
        ):
            # ---- device projections + table assembly (AllGather of kv blocks) ----
            agin = dram.tile([NPTS, 128], bf16)
            tbl = dram.tile([N, 128], bf16)
            wsp = dram.tile([64, T], bf16)
            gsp = dram.tile([64, T], bf16)
            w1sp = dram.tile([8, T], bf16)
            ar0i = dram.tile([64, 2], f32)
            ar0o = dram.tile([64, 2], f32)
            ar1i = dram.tile([8, 2], f32)
            ar1o = dram.tile([8, 2], f32)

            # ---- small persistent SBUF: weights, params, stats ----
            w3_sb = prm.tile([4, 128], bf16)
            nc.gpsimd.dma_start(w3_sb[:], w3[:, :])
            b128_sb = prm.tile([128, 1], f32)
            nc.gpsimd.dma_start(b128_sb[:], b128[:, :])
            w64_sb = prm.tile([64, 8], bf16)
            nc.gpsimd.dma_start(w64_sb[:], w64[:, :])
            w8_sb = prm.tile([8, 64], bf16)
            nc.gpsimd.dma_start(w8_sb[:], w8[:, :])
            b64_sb = prm.tile([64, 5], f32)
            nc.gpsimd.dma_start(b64_sb[:], b64[:, :])
            b8_sb = prm.tile([8, 4], f32)
            nc.gpsimd.dma_start(b8_sb[:], b8[:, :])
            xq_sb = prm.tile([64, NPTS], bf16)
            idx_sb = prm.tile([128, NCHUNK * G * ICOL], i16)
            nc.gpsimd.dma_start(idx_sb[0:16, :], idx16[:, :])
            nc.gpsimd.dma_start(idx_sb[16:32, :], idx_sb[0:16, :])
            nc.gpsimd.dma_start(idx_sb[32:64, :], idx_sb[0:32, :])
            nc.gpsimd.dma_start(idx_sb[64:128, :], idx_sb[0:64, :])

            with (
                tc.tile_pool(name="xap", bufs=1) as xap,
                tc.tile_pool(name="kst", bufs=2) as kst,
                tc.tile_pool(name="ps0", bufs=2, space=bass.MemorySpace.PSUM) as psT,
            ):
                xa = xap.tile([65, NPTS], bf16)
                nc.gpsimd.dma_start(xa[0:64, :], xT[:, :])
                nc.gpsimd.memset(xa[64:65, :], 1.0)
                wkv_sb = xap.tile([65, 128], bf16)
                nc.gpsimd.dma_start(wkv_sb[:], wkvb[:, :])
                wq_sb = xap.tile([65, 64], bf16)
                nc.gpsimd.dma_start(wq_sb[:], wqb[:, :])
                # x_q^T (channel-major) for the own shard
                QS = min(512, NPTS)
                for s in range(NPTS // QS):
                    pq = psT.tile([64, QS], f32)
                    nc.tensor.matmul(pq[:], wq_sb[:], xa[:, s * QS:(s + 1) * QS])
                    nc.scalar.activation(xq_sb[:, s * QS:(s + 1) * QS], pq[:],
                                         AF.Identity)
                # kv rows (point-major) for the own shard -> AllGather -> table
                T128 = NPTS // 128
                GRP = 4 if T128 % 4 == 0 else (2 if T128 % 2 == 0 else 1)
                for t4 in range(T128 // GRP):
                    pk = psT.tile([128, GRP * 128], f32)
                    for t in range(GRP):
                        nc.tensor.matmul(
                            pk[:].rearrange("p (a b) -> p a b", a=GRP)[:, t, :],
                            xa[:, (t4 * GRP + t) * 128:(t4 * GRP + t + 1) * 128],
                            wkv_sb[:])
                    kv4 = kst.tile([128, GRP * 128], bf16)
                    nc.scalar.activation(kv4[:], pk[:], AF.Identity)
                    nc.gpsimd.dma_start(
                        agin[t4 * GRP * 128:(t4 + 1) * GRP * 128, :].rearrange(
                            "(a b) c -> b a c", b=128),
                        kv4[:].rearrange("p (a b) -> p a b", a=GRP))
            nc.gpsimd.collective_compute(
                "AllGather", OP.bypass,
                replica_groups=[list(range(NC))],
                ins=[agin.opt()], outs=[tbl.opt()],
            )

            t1c = prm.tile([4, CHUNK], bf16)
            nc.gpsimd.memset(t1c[:], 0)
            sA = prm.tile([64, NCHUNK], f32)    # per-chunk sums of w_pre
            sAq = prm.tile([64, NCHUNK], f32)   # per-chunk sumsq of w_pre
            sB = prm.tile([8, NCHUNK], f32)
            sBq = prm.tile([8, NCHUNK], f32)
            st0 = prm.tile([64, 2], f32)
            st1 = prm.tile([8, 2], f32)
            sc0 = prm.tile([64, 1], f32)        # bn0 scale
            sh0 = prm.tile([64, 1], f32)        # bn0 shift
            sc1 = prm.tile([8, 1], f32)
            sh1 = prm.tile([8, 1], f32)
            tmp0 = prm.tile([64, 4], f32)
            tmp1 = prm.tile([8, 4], f32)

            # ================= phase A: gather + w_pre + gvpr + stats =========
            with (
                tc.tile_pool(name="kvp", bufs=2) as kvp,
                tc.tile_pool(name="prp", bufs=1) as prp,
                tc.tile_pool(name="wgp", bufs=2) as wgp,
                tc.tile_pool(name="scp", bufs=1) as scp,
                tc.tile_pool(name="ps1", bufs=2, space=bass.MemorySpace.PSUM) as psA1,
            ):
                for c in range(NCHUNK):
                    kv = kvp.tile([128, G * GN], bf16)
                    for g in range(G):
                        nc.gpsimd.dma_gather(
                            kv[:, g * GN:(g + 1) * GN].rearrange("p (a b) -> p a b", a=1),
                            tbl[N // 2:, :],
                            idx_sb[:, (c * G + g) * ICOL:(c * G + g + 1) * ICOL],
                            GN, GN, 128,
                            transpose=True,
                        )
                    nc.gpsimd.dma_start(t1c[0:3, :], t1nT[:, c * CHUNK:(c + 1) * CHUNK])
                    pr = prp.tile([128, CHUNK], bf16)
                    for q in range(SUB // 4):
                        ps = psA1.tile([128, 2048], f32)
                        for s in range(4):
                            nc.tensor.matmul(
                                ps[:, s * 512:(s + 1) * 512], w3_sb[:],
                                t1c[:, q * 2048 + s * 512:q * 2048 + (s + 1) * 512])
                        nc.scalar.activation(pr[:, q * 2048:(q + 1) * 2048], ps[:],
                                             AF.Identity, bias=b128_sb[:])
                    # wg[0:64] = w_pre = g_k - q_bcast + p_r ; wg[64:128] = g_v + p_r
                    wg = wgp.tile([128, CHUNK], bf16)
                    kv_k4 = (kv[0:64, :].rearrange("p (g r) -> p g r", g=G)[:, :, 0:512]
                             .rearrange("p g (i j) -> p g i j", j=NS))
                    qb = (xq_sb[:, c * PPC:(c + 1) * PPC]
                          .rearrange("p (g i) -> p g i", g=G)
                          .unsqueeze(3).broadcast_to([64, G, 512 // NS, NS]))
                    nc.vector.tensor_tensor(
                        wg[0:64, :].rearrange("p (g i j) -> p g i j", g=G, j=NS),
                        kv_k4, qb, OP.subtract)
                    nc.vector.tensor_tensor(wg[0:64, :], wg[0:64, :], pr[0:64, :],
                                            OP.add)
                    kv_v3 = kv[64:128, :].rearrange("p (g r) -> p g r", g=G)[:, :, 0:512]
                    nc.vector.tensor_tensor(
                        wg[64:128, :].rearrange("p (g r) -> p g r", g=G),
                        kv_v3, pr[64:128, :].rearrange("p (g r) -> p g r", g=G), OP.add)
                    # stats
                    nc.vector.reduce_sum(sA[:, c:c + 1], wg[0:64, :], axis=AX)
                    scr = scp.tile([64, CHUNK], bf16)
                    nc.scalar.activation(scr[:], wg[0:64, :], AF.Square,
                                         accum_out=sAq[:, c:c + 1])
                    nc.gpsimd.dma_start(wsp[:, c * CHUNK:(c + 1) * CHUNK], wg[0:64, :])
                    nc.gpsimd.dma_start(gsp[:, c * CHUNK:(c + 1) * CHUNK], wg[64:128, :])

            # ---- bn0 stats all-reduce ----
            nc.vector.reduce_sum(st0[:, 0:1], sA[:], axis=AX)
            nc.vector.reduce_sum(st0[:, 1:2], sAq[:], axis=AX)
            nc.gpsimd.dma_start(ar0i[:], st0[:])
            nc.gpsimd.collective_compute(
                "AllReduce", OP.add, replica_groups=[list(range(NC))],
                ins=[ar0i.opt()], outs=[ar0o.opt()])
            nc.gpsimd.dma_start(st0[:], ar0o[:])
            inv = 1.0 / NT
            nc.scalar.mul(tmp0[:, 0:1], st0[:, 0:1], inv)            # mean
            nc.scalar.mul(tmp0[:, 1:2], st0[:, 1:2], inv)            # E[x^2]
            nc.scalar.activation(tmp0[:, 2:3], tmp0[:, 0:1], AF.Square)
            nc.vector.tensor_tensor(tmp0[:, 1:2], tmp0[:, 1:2], tmp0[:, 2:3],
                                    OP.subtract)                     # var
            nc.vector.tensor_tensor(tmp0[:, 1:2], tmp0[:, 1:2], b64_sb[:, 4:5], OP.add)
            nc.scalar.activation(tmp0[:, 2:3], tmp0[:, 1:2], AF.Sqrt)
            nc.vector.reciprocal(tmp0[:, 3:4], tmp0[:, 2:3])         # rstd
            nc.vector.tensor_tensor(sc0[:], b64_sb[:, 1:2], tmp0[:, 3:4], OP.mult)
            nc.vector.tensor_tensor(tmp0[:, 2:3], tmp0[:, 0:1], sc0[:], OP.mult)
            nc.vector.tensor_tensor(sh0[:], b64_sb[:, 2:3], tmp0[:, 2:3], OP.subtract)

            # ================= phase B: w1 = relu(bn0(w_pre)) @ Ww1.T + bw1 ====
            with (
                tc.tile_pool(name="wpl", bufs=2) as wpl,
                tc.tile_pool(name="wrp", bufs=1) as wrp,
                tc.tile_pool(name="w1p", bufs=2) as w1p,
                tc.tile_pool(name="sc8", bufs=1) as sc8,
                tc.tile_pool(name="ps2", bufs=2, space=bass.MemorySpace.PSUM) as psB,
            ):
                for c in range(NCHUNK):
                    wp = wpl.tile([64, CHUNK], bf16)
                    nc.gpsimd.dma_start(wp[:], wsp[:, c * CHUNK:(c + 1) * CHUNK])
                    wr = wrp.tile([64, CHUNK], bf16)
                    nc.scalar.activation(wr[:], wp[:], AF.Relu, scale=sc0[:], bias=sh0[:])
                    w1 = w1p.tile([8, CHUNK], bf16)
                    for q in range(SUB // 4):
                        ps = psB.tile([8, 2048], f32)
                        for s in range(4):
                            nc.tensor.matmul(
                                ps[:, s * 512:(s + 1) * 512], w64_sb[:],
                                wr[:, q * 2048 + s * 512:q * 2048 + (s + 1) * 512])
                        nc.scalar.activation(w1[:, q * 2048:(q + 1) * 2048], ps[:],
                                             AF.Identity, bias=b8_sb[:, 0:1])
                    nc.vector.reduce_sum(sB[:, c:c + 1], w1[:], axis=AX)
                    scr = sc8.tile([8, CHUNK], bf16)
                    nc.scalar.activation(scr[:], w1[:], AF.Square,
                                         accum_out=sBq[:, c:c + 1])
                    nc.gpsimd.dma_start(w1sp[:, c * CHUNK:(c + 1) * CHUNK], w1[:])

            # ---- bn1 stats all-reduce ----
            nc.vector.reduce_sum(st1[:, 0:1], sB[:], axis=AX)
            nc.vector.reduce_sum(st1[:, 1:2], sBq[:], axis=AX)
            nc.gpsimd.dma_start(ar1i[:], st1[:])
            nc.gpsimd.collective_compute(
                "AllReduce", OP.add, replica_groups=[list(range(NC))],
                ins=[ar1i.opt()], outs=[ar1o.opt()])
            nc.gpsimd.dma_start(st1[:], ar1o[:])
            nc.scalar.mul(tmp1[:, 0:1], st1[:, 0:1], inv)
            nc.scalar.mul(tmp1[:, 1:2], st1[:, 1:2], inv)
            nc.scalar.activation(tmp1[:, 2:3], tmp1[:, 0:1], AF.Square)
            nc.vector.tensor_tensor(tmp1[:, 1:2], tmp1[:, 1:2], tmp1[:, 2:3],
                                    OP.subtract)
            nc.vector.tensor_tensor(tmp1[:, 1:2], tmp1[:, 1:2], b8_sb[:, 3:4], OP.add)
            nc.scalar.activation(tmp1[:, 2:3], tmp1[:, 1:2], AF.Sqrt)
            nc.vector.reciprocal(tmp1[:, 3:4], tmp1[:, 2:3])
            nc.vector.tensor_tensor(sc1[:], b8_sb[:, 1:2], tmp1[:, 3:4], OP.mult)
            nc.vector.tensor_tensor(tmp1[:, 2:3], tmp1[:, 0:1], sc1[:], OP.mult)
            nc.vector.tensor_tensor(sh1[:], b8_sb[:, 2:3], tmp1[:, 2:3], OP.subtract)

            # ========== phase C: w2, softmax over neighbors, aggregate ========
            with (
                tc.tile_pool(name="w1l", bufs=2) as w1l,
                tc.tile_pool(name="w1n", bufs=1) as w1np,
                tc.tile_pool(name="ep", bufs=1) as ep,
                tc.tile_pool(name="gvl", bufs=2) as gvl,
                tc.tile_pool(name="rp", bufs=1) as rp,
                tc.tile_pool(name="pp", bufs=1) as pp,
                tc.tile_pool(name="sop", bufs=2) as sop,
                tc.tile_pool(name="ps3", bufs=2, space=bass.MemorySpace.PSUM) as psC,
            ):
                for c in range(NCHUNK):
                    w1 = w1l.tile([8, CHUNK], bf16)
                    nc.gpsimd.dma_start(w1[:], w1sp[:, c * CHUNK:(c + 1) * CHUNK])
                    w1n = w1np.tile([8, CHUNK], bf16)
                    nc.scalar.activation(w1n[:], w1[:], AF.Relu, scale=sc1[:], bias=sh1[:])
                    e = ep.tile([64, CHUNK], bf16)
                    for q in range(SUB // 4):
                        ps = psC.tile([64, 2048], f32)
                        for s in range(4):
                            nc.tensor.matmul(
                                ps[:, s * 512:(s + 1) * 512], w8_sb[:],
                                w1n[:, q * 2048 + s * 512:q * 2048 + (s + 1) * 512])
                        nc.scalar.activation(e[:, q * 2048:(q + 1) * 2048], ps[:],
                                             AF.Exp, bias=b64_sb[:, 3:4])
                    sav = sop.tile([64, PPC], f32)
                    nc.vector.reduce_sum(sav[:], e[:].rearrange("p (i j) -> p i j", j=NS),
                                         axis=AX)
                    rcp = sop.tile([64, PPC], f32)
                    nc.vector.reciprocal(rcp[:], sav[:])
                    gv = gvl.tile([64, CHUNK], bf16)
                    nc.gpsimd.dma_start(gv[:], gsp[:, c * CHUNK:(c + 1) * CHUNK])
                    r = rp.tile([64, CHUNK], bf16)
                    nc.vector.tensor_tensor(
                        r[:].rearrange("p (i j) -> p i j", j=NS),
                        e[:].rearrange("p (i j) -> p i j", j=NS),
                        rcp[:].unsqueeze(2).broadcast_to([64, PPC, NS]), OP.mult)
                    pm = pp.tile([64, CHUNK], bf16)
                    nc.vector.tensor_tensor(pm[:], r[:], gv[:], OP.mult)
                    o32 = sop.tile([64, PPC], f32)
                    nc.vector.reduce_sum(o32[:], pm[:].rearrange("p (i j) -> p i j", j=NS),
                                         axis=AX)
                    o = sop.tile([64, PPC], bf16)
                    nc.scalar.copy(o[:], o32[:])
                    nc.gpsimd.dma_start(outT[:, c * PPC:(c + 1) * PPC], o[:])

    nc.compile()
    _cache[cfg_key] = (nc, cfg)
    return nc, cfg


def _host_prep(cfg, p, x, idx, Wq, bq, Wk, bk, Wv, bv, Wp1, bp1, bn_p_g, bn_p_b,
               Wp2, bp2, bn_w0_g, bn_w0_b, Ww1, bw1, bn_w1_g, bn_w1_b, Ww2, bw2):
    N, NS, NC, CHUNK = cfg["N"], cfg["NS"], cfg["NC"], cfg["CHUNK"]
    NPTS = N // NC
    T = NPTS * NS
    G = CHUNK // 512
    GN, ICOL = 640, 40

    f = np.float32
    p = np.asarray(p, f); x = np.asarray(x, f); idx = np.asarray(idx)
    xbf = x.astype(ml_dtypes.bfloat16)
    wkvb = np.concatenate([
        np.concatenate([np.asarray(Wk, f).T, np.asarray(Wv, f).T], axis=1),
        np.concatenate([np.asarray(bk, f), np.asarray(bv, f)])[None, :],
    ], axis=0).astype(ml_dtypes.bfloat16)
    wqb = np.concatenate([np.asarray(Wq, f).T, np.asarray(bq, f)[None, :]],
                         axis=0).astype(ml_dtypes.bfloat16)

    # position front: t1n = relu(bn_p(g_p @ Wp1.T + bp1)), exact fp32 on host
    n, ns = idx.shape
    g_p = (p[idx.reshape(-1)] - np.repeat(p, ns, axis=0)).reshape(n, ns, 3)
    t1 = (g_p.reshape(-1, 3) @ np.asarray(Wp1, f).T).reshape(n, ns, 3) \
        + np.asarray(bp1, f)
    m = t1.mean(axis=(0, 1)); v = t1.var(axis=(0, 1))
    t1n = np.maximum((t1 - m) / np.sqrt(v + EPS) * np.asarray(bn_p_g, f)
                     + np.asarray(bn_p_b, f), 0.0)

    # signed gather offsets relative to base N//2 (natural-order table);
    # pad index 0 in the packed buffer means row N//2 — a valid in-range row
    off = (idx.astype(np.int64) - N // 2).astype(np.int16)

    w3 = np.zeros((4, 128), ml_dtypes.bfloat16)
    w3[:3] = np.tile(np.asarray(Wp2, f).T, (1, 2)).astype(ml_dtypes.bfloat16)
    b128 = np.tile(np.asarray(bp2, f), 2).reshape(128, 1).astype(f)
    w64 = np.asarray(Ww1, f).T.astype(ml_dtypes.bfloat16)
    w8 = np.tile(np.asarray(Ww2, f).T, (1, 8)).astype(ml_dtypes.bfloat16)
    b64 = np.stack([np.asarray(bp2, f), np.asarray(bn_w0_g, f),
                    np.asarray(bn_w0_b, f), np.tile(np.asarray(bw2, f), 8),
                    np.full(64, EPS, f)], axis=1).astype(f)
    b8 = np.stack([np.asarray(bw1, f), np.asarray(bn_w1_g, f),
                   np.asarray(bn_w1_b, f), np.full(8, EPS, f)], axis=1).astype(f)

    in_maps = []
    for c in range(NC):
        lo, hi = c * NPTS, (c + 1) * NPTS
        offc = off[lo:hi].reshape(-1)                    # [T]
        packed = np.zeros((T // 512, GN), np.int16)
        packed[:, :512] = offc.reshape(T // 512, 512)
        idx16 = packed.reshape(T // 512 * ICOL, 16).T.copy()   # [16, cols]
        t1nT = np.ascontiguousarray(
            t1n[lo:hi].reshape(T, 3).T).astype(ml_dtypes.bfloat16)
        in_maps.append({
            "xT": np.ascontiguousarray(xbf[lo:hi].T),
            "wkvb": wkvb, "wqb": wqb,
            "t1nT": t1nT,
            "idx16": idx16,
            "w3": w3, "w64": w64, "w8": w8, "b64": b64, "b8": b8,
            "b128": b128,
        })
    return in_maps


_spmd_cache = {}


def _spmd_cached(nc, in_maps, n_cores):
    """run_bass_via_pjrt's multi-core path with the traced/jitted executable
    cached across calls (the library re-traces each invocation)."""
    import jax
    from jax.experimental.shard_map import shard_map
    from jax.sharding import Mesh, PartitionSpec
    from concourse import bass2jax as b2j
    import concourse.mybir as _mb

    key = id(nc)
    if key not in _spmd_cache:
        b2j.install_neuronx_cc_hook()
        assert nc.dbg_addr is None
        partition_name = (nc.partition_id_tensor.name
                          if nc.partition_id_tensor else None)
        in_names, out_names, out_avals = [], [], []
        for alloc in nc.m.functions[0].allocations:
            if not isinstance(alloc, _mb.MemoryLocationSet):
                continue
            name = alloc.memorylocations[0].name
            if alloc.kind == "ExternalInput":
                if name != partition_name:
                    in_names.append(name)
            elif alloc.kind == "ExternalOutput":
                out_names.append(name)
                out_avals.append(jax.core.ShapedArray(
                    tuple(alloc.tensor_shape), _mb.dt.np(alloc.dtype)))
        n_params = len(in_names)
        all_names = in_names + out_names
        if partition_name is not None:
            all_names.append(partition_name)

        def _body(*args):
            operands = list(args)
            if partition_name is not None:
                operands.append(b2j.partition_id_tensor())
            return tuple(b2j._bass_exec_p.bind(
                *operands,
                out_avals=tuple(out_avals),
                in_names=tuple(all_names),
                out_names=tuple(out_names),
                lowering_input_output_aliases=(),
                sim_require_finite=True,
                sim_require_nnan=True,
                nc=nc,
            ))

        devices = jax.devices()[:n_cores]
        mesh = Mesh(np.asarray(devices), ("core",))
        n_outs = len(out_names)
        sharded = jax.jit(
            shard_map(_body, mesh=mesh,
                      in_specs=(PartitionSpec("core"),) * (n_params + n_outs),
                      out_specs=(PartitionSpec("core"),) * n_outs,
                      check_rep=False),
            donate_argnums=tuple(range(n_params, n_params + n_outs)),
            keep_unused=True,
        )
        _spmd_cache[key] = (sharded, in_names, out_names, out_avals, n_params)

    sharded, in_names, out_names, out_avals, n_params = _spmd_cache[key]
    concat_in = [
        np.concatenate([np.asarray(m[name]) for m in in_maps], axis=0)
        for name in in_names
    ]
    concat_zeros = [
        np.zeros((n_cores * a.shape[0], *a.shape[1:]), a.dtype)
        for a in out_avals
    ]
    out_arrs = sharded(*concat_in, *concat_zeros)
    return [
        {name: np.asarray(out_arrs[i]).reshape(n_cores, *out_avals[i].shape)[c]
         for i, name in enumerate(out_names)}
        for c in range(n_cores)
    ]


def _run(cfg_key, inputs):
    nc, cfg = _build(cfg_key)
    in_maps = _host_prep(cfg, **inputs)
    try:
        results = _spmd_cached(nc, in_maps, cfg["NC"])
    except Exception:
        results = run_bass_kernel_spmd(nc, in_maps, list(range(cfg["NC"]))).results
    N, NC = cfg["N"], cfg["NC"]
    NPTS = N // NC
    out = np.empty((N, 64), np.float32)
    for c in range(NC):
        out[c * NPTS:(c + 1) * NPTS] = results[c]["outT"].astype(np.float32).T
    return out


def kernel(p, x, idx, **kw):
    inputs = dict(p=p, x=x, idx=idx, **kw)
    return _run("full", inputs)
